# revision 45
# baseline (speedup 1.0000x reference)
"""Bag self-attention kernel for TRN2, data-parallel over the bag dim (8 cores).

Per core (one bag, x: [N=2048, L=1280], H=160):
  q = x@Wq.T + bq ; k = x@Wk.T (bk cancels in softmax) ; v = x@Wv.T
  S = q@k.T ; P = softmax(S) ; out = P@v + (x + bv)      (gamma = 1)

Device-side strategy (fp8-DoubleRow accelerated):
  - q/k projections and S = q@k^T run in float32r (full PE rate at free>=256).
  - S is computed in [i, j] orientation so the per-row max is a free-dim
    reduce; exp(S - max) is an activation with a per-partition bias and
    writes E in bf16. E blocks are PE-transposed (bf16 identity) to [j, i]
    and cast to fp8e4 during the PSUM->SBUF copy.
  - v-projection runs as 3-term compensated fp8 DoubleRow at 32x scale:
    v32 = xh@Wh32 + xh@We32 + xe@Wh32 accumulated in one PSUM group, where
    xh/xe (fp8 value + fp8 error of x) and Wh32/We32 (fp8 of 32*WvT and its
    fp8 error) are prepared host-side. 1.5 rows per 256-contraction instead
    of fp32r's 2.0.
  - v32 is split on-device into fp8 planes (vh, ve = v32 - vh), and the
    attention output is a DoubleRow matmul with planes (E, E) x (vh, ve),
    i.e. E@(vh+ve): v at ~16-bit effective precision, 2x the fp32r rate.
    The stationary E plane pair is a stride-0 broadcast AP (no duplicate).
  - The softmax normalizer rides the last P@v m-chunk via a constant
    column (32, 0) in the v planes: Z32 = sum_j E8 * 32, so the 32x scale
    on v cancels exactly in out = num * (1/Z32-col) and the normalization
    uses the same quantized E8 as the numerator (first-order quantization
    cancellation).
    The 32x scale keeps v32 under fp8e4's 240 max finite (the hw fp8e4 is
    IEEE e4m3 WITH infinities - overflow turns into inf, then NaN).
  - bv rides the residual (sum_j P == 1); bk drops (row-constant energy).
"""

import numpy as np
import ml_dtypes

import concourse.mybir as mybir
import concourse.tile as tile
from concourse import bacc
from concourse.bass_utils import run_bass_kernel_spmd

B, N, L, H = 8, 2048, 1280, 160
f32 = mybir.dt.float32
f32r = mybir.dt.float32r
bf16 = mybir.dt.bfloat16
f8 = mybir.dt.float8e4
DR = mybir.MatmulPerfMode.DoubleRow
E4 = ml_dtypes.float8_e4m3

NL = L // 128            # 10 l-chunks (f32r projection contraction)
NP = NL // 2             # 5 l-pairs (fp8 DoubleRow contraction)
NJ = N // 128            # 16 token blocks (j)
NIT = N // 128           # 16 i-tiles
JC = N // 512            # 4 j-chunks for S
MC = 5                   # m-chunks of 256
H0, H1 = 128, H - 128    # 128 + 32


def _build():
    nc = bacc.Bacc()
    xT_d = nc.declare_dram_parameter("xT", [128, NL, N], f32r, isOutput=False)
    xh_d = nc.declare_dram_parameter("xh", [128, NP, 2, N], f8, isOutput=False)
    xe_d = nc.declare_dram_parameter("xe", [128, NP, 2, N], f8, isOutput=False)
    # weight layouts pre-arranged host-side for contiguous per-partition DMA
    wq_d = nc.declare_dram_parameter("WqT", [128, NL, H0], f32r, isOutput=False)
    wk_d = nc.declare_dram_parameter("WkT", [128, NL, H0], f32r, isOutput=False)
    wqk1_d = nc.declare_dram_parameter("Wqk1T", [128, NL, 2 * H1], f32r,
                                       isOutput=False)
    wh_d = nc.declare_dram_parameter("Wh64", [MC, 128, NP, 2, 256], f8,
                                     isOutput=False)
    we_d = nc.declare_dram_parameter("We64", [MC, 128, NP, 2, 256], f8,
                                     isOutput=False)
    bq_d = nc.declare_dram_parameter("bq", [H], f32, isOutput=False)
    id_d = nc.declare_dram_parameter("ident", [128, 128], bf16, isOutput=False)
    xr_d = nc.declare_dram_parameter("xresid", [N, L], f32, isOutput=False)
    out_d = nc.declare_dram_parameter("out", [N, L], f32, isOutput=True)

    with tile.TileContext(nc) as tc:
        with (
            tc.tile_pool(name="const", bufs=1) as constp,
            tc.tile_pool(name="vpl", bufs=1) as vpool,
            tc.tile_pool(name="qkt", bufs=1) as qktp,
        ):
            bq_t = [constp.tile([H0, 1], f32, tag="bq0", name="bq0"),
                    constp.tile([H1, 1], f32, tag="bq1", name="bq1")]
            id_t = constp.tile([128, 128], bf16, tag="ident", name="ident")

            # v planes: [tok 128, (vh, ve), m 1280 + Z-col + pad] fp8 per token
            # block. Col L holds (32, 0) so the softmax normalizer Z*32 rides
            # the same accumulation group as the last P@v m-chunk.
            v_t = [vpool.tile([128, 2, L + 2], f8, tag=f"v{j}", name=f"v{j}")
                   for j in range(NJ)]

            # q/k resident: [h, N] f32r
            qT0 = qktp.tile([H0, N], f32r, tag="q0", name="q0")
            kT0 = qktp.tile([H0, N], f32r, tag="k0", name="k0")
            qT1 = qktp.tile([H1, N], f32r, tag="q1", name="q1")
            kT1 = qktp.tile([H1, N], f32r, tag="k1", name="k1")

            ebfp = tc.alloc_tile_pool(name="ebf", bufs=3)
            pwp = tc.alloc_tile_pool(name="pw", bufs=1)
            e_bf = {}

            # ---- Phase 1: projections, 4 token generations of 512
            with (
                tc.tile_pool(name="wqk", bufs=1) as wqkp,
                tc.tile_pool(name="w8", bufs=1) as w8p,
                tc.tile_pool(name="xt", bufs=2, side="right") as xtp,
                tc.tile_pool(name="x8", bufs=2, side="right") as x8p,
                tc.tile_pool(name="vps", bufs=4, space="PSUM") as vps,
                tc.tile_pool(name="qkps", bufs=1, space="PSUM") as qkps,
                tc.tile_pool(name="spre", bufs=1, space="PSUM") as sprep,
            ):
                def prewarm_s_chunk(it, jc, sbs, mx4):
                    # S + row-max chunk for an early i-tile, staged via SBUF
                    # so it needs only one psum bank alongside the P1 pools
                    itsl = slice(it * 128, (it + 1) * 128)
                    jsl = slice(jc * 512, (jc + 1) * 512)
                    ps = sprep.tile([128, 512], f32, tag="spre",
                                    name=f"spre{it}_{jc}")
                    nc.tensor.matmul(ps, qT0[:, itsl], kT0[:, jsl],
                                     start=True, stop=False)
                    nc.tensor.matmul(ps, qT1[:, itsl], kT1[:, jsl],
                                     start=False, stop=True)
                    nc.scalar.activation(sbs[:, jsl], ps,
                                         mybir.ActivationFunctionType.Copy)
                    nc.vector.tensor_reduce(
                        mx4[:, jc:jc + 1], ps, mybir.AxisListType.X,
                        mybir.AluOpType.max)

                def prewarm_exp(it, sbs, nm):
                    ebf = ebfp.tile([128, N], bf16, tag="ebf",
                                    name=f"ebf{it}")
                    for jc in range(JC):
                        jsl = slice(jc * 512, (jc + 1) * 512)
                        nc.scalar.activation(ebf[:, jsl], sbs[:, jsl],
                                             mybir.ActivationFunctionType.Exp,
                                             bias=nm)
                    e_bf[it] = ebf
                def dma_x_gen(g):
                    # one tile, two half-DMAs (overlaps other critical loads)
                    c0 = g * 512
                    t = xtp.tile([128, NL, 512], f32r, tag="x", name=f"x{g}")
                    nc.sync.dma_start(out=t[:, :, 0:256],
                                      in_=xT_d[:, :, c0:c0 + 256])
                    nc.sync.dma_start(out=t[:, :, 256:512],
                                      in_=xT_d[:, :, c0 + 256:c0 + 512])
                    return t

                def dma_x8_gen(g):
                    c0 = g * 512
                    th = x8p.tile([128, NP, 2, 512], f8, tag="xh",
                                  name=f"xh{g}")
                    nc.sync.dma_start(out=th, in_=xh_d[:, :, :, c0:c0 + 512])
                    te = x8p.tile([128, NP, 2, 512], f8, tag="xe",
                                  name=f"xe{g}")
                    nc.sync.dma_start(out=te, in_=xe_d[:, :, :, c0:c0 + 512])
                    return th, te

                def qk_proj(g, x_t):
                    isl = slice(g * 512, (g + 1) * 512)
                    ps3 = qkps.tile([2 * H1, 512], f32, tag="qk1ps",
                                    name=f"qk1ps{g}")
                    for l in range(NL):
                        nc.tensor.matmul(ps3, wqk1_t[:, l, :], x_t[:, l, :],
                                         start=(l == 0), stop=(l == NL - 1))
                    nc.vector.tensor_scalar_add(qT1[:, isl], ps3[0:H1, :], bq_t[1])
                    nc.any.tensor_copy(kT1[:, isl], ps3[H1:2 * H1, :])
                    ps = qkps.tile([H0, 512], f32, tag="qps", name=f"qps{g}")
                    for l in range(NL):
                        nc.tensor.matmul(ps, wq_t[:, l, :], x_t[:, l, :],
                                         start=(l == 0), stop=(l == NL - 1))
                    nc.vector.tensor_scalar_add(qT0[:, isl], ps, bq_t[0])
                    ps2 = qkps.tile([H0, 512], f32, tag="kps", name=f"kps{g}")
                    for l in range(NL):
                        nc.tensor.matmul(ps2, wk_t[:, l, :], x_t[:, l, :],
                                         start=(l == 0), stop=(l == NL - 1))
                    nc.any.tensor_copy(kT0[:, isl], ps2)

                def v_proj(g, xh_t, xe_t, hook=None):
                    for mc in range(MC):
                        if hook is not None:
                            hook(mc)
                        msl = slice(mc * 256, (mc + 1) * 256)
                        for t in range(4):
                            j = 4 * g + t
                            tsl = slice(t * 128, (t + 1) * 128)
                            ps = vps.tile([128, 256], f32, tag="vps",
                                          name=f"vps{g}_{mc}_{t}")
                            for p in range(NP):
                                nc.tensor.matmul(
                                    ps, xh_t[:, p, :, tsl], wh_t[:, mc, p],
                                    start=(p == 0), stop=False, perf_mode=DR)
                            for p in range(NP):
                                nc.tensor.matmul(
                                    ps, xh_t[:, p, :, tsl], we_t[:, mc, p],
                                    start=False, stop=False, perf_mode=DR)
                            for p in range(NP):
                                nc.tensor.matmul(
                                    ps, xe_t[:, p, :, tsl], wh_t[:, mc, p],
                                    start=False, stop=(p == NP - 1), perf_mode=DR)
                            # vh = fp8(v64); ve = fp8(v64 - vh)
                            if t % 2 == 0:
                                nc.vector.tensor_copy(v_t[j][:, 0, msl], ps)
                            else:
                                nc.scalar.activation(
                                    v_t[j][:, 0, msl], ps,
                                    mybir.ActivationFunctionType.Copy)
                            nc.vector.tensor_tensor(
                                out=v_t[j][:, 1, msl], in0=ps,
                                in1=v_t[j][:, 0, msl],
                                op=mybir.AluOpType.subtract)

                # DMA order: v_proj(0)'s operands first (fp8, small), then the
                # f32r q/k path loads while v_proj(0) runs on the PE.
                # DMA queue order tuned against per-chunk need-by times:
                # xe0 rides after the first weight chunk (the xe term is the
                # last of the three in each psum group)
                th0 = x8p.tile([128, NP, 2, 512], f8, tag="xh", name="xh0")
                nc.sync.dma_start(out=th0, in_=xh_d[:, :, :, 0:512])
                wh_t = w8p.tile([128, MC, NP, 2, 256], f8, tag="wh", name="wh")
                we_t = w8p.tile([128, MC, NP, 2, 256], f8, tag="we", name="we")

                def dma_w8(mc):
                    nc.sync.dma_start(out=wh_t[:, mc], in_=wh_d[mc])
                    nc.sync.dma_start(out=we_t[:, mc], in_=we_d[mc])

                dma_w8(0)
                te0 = x8p.tile([128, NP, 2, 512], f8, tag="xe", name="xe0")
                nc.sync.dma_start(out=te0, in_=xe_d[:, :, :, 0:512])
                xh_cur = (th0, te0)
                dma_w8(1)
                wqk1_t = wqkp.tile([128, NL, 2 * H1], f32r, tag="wqk1",
                                   name="wqk1")
                nc.sync.dma_start(out=wqk1_t, in_=wqk1_d[:, :, :])
                x_cur = xtp.tile([128, NL, 512], f32r, tag="x", name="x0")
                nc.sync.dma_start(out=x_cur[:, :, 0:256], in_=xT_d[:, :, 0:256])
                dma_w8(2)
                dma_w8(3)
                dma_w8(4)
                xh1t = x8p.tile([128, NP, 2, 512], f8, tag="xh", name="xh1")
                nc.sync.dma_start(out=xh1t, in_=xh_d[:, :, :, 512:1024])
                nc.sync.dma_start(out=x_cur[:, :, 256:512],
                                  in_=xT_d[:, :, 256:512])
                xe1t = x8p.tile([128, NP, 2, 512], f8, tag="xe", name="xe1")
                nc.sync.dma_start(out=xe1t, in_=xe_d[:, :, :, 512:1024])
                xh_next = (xh1t, xe1t)
                wq_t = wqkp.tile([128, NL, H0], f32r, tag="wq", name="wq")
                nc.sync.dma_start(out=wq_t, in_=wq_d[:, :, :])
                wk_t = wqkp.tile([128, NL, H0], f32r, tag="wk", name="wk")
                nc.sync.dma_start(out=wk_t, in_=wk_d[:, :, :])
                nc.sync.dma_start(out=bq_t[0], in_=bq_d[0:H0].unsqueeze(1))
                nc.sync.dma_start(out=bq_t[1], in_=bq_d[H0:H].unsqueeze(1))
                nc.sync.dma_start(out=id_t, in_=id_d[:, :])
                for j in range(NJ):
                    nc.vector.memset(v_t[j][:, :, L:L + 2], 0.0)
                    nc.vector.memset(v_t[j][:, 0, L:L + 1], 32.0)

                v_proj(0, *xh_cur)
                x_next = dma_x_gen(1)
                qk_proj(0, x_cur)
                for g in (1, 2):
                    x_cur, xh_cur = x_next, xh_next
                    xh_next = dma_x8_gen(g + 1)
                    v_proj(g, *xh_cur)
                    x_next = dma_x_gen(g + 1)
                    qk_proj(g, x_cur)
                # gen 3: projections first, then pre-warm the first two
                # attention i-tiles while v_proj(3) keeps the PE busy
                qk_proj(3, x_next)
                sbs0 = pwp.tile([128, N], f32, tag="sbs0", name="sbs0")
                mx40 = pwp.tile([128, 4], f32, tag="mx4p0", name="mx4p0")
                v_proj(3, *xh_next,
                       hook=lambda mc: prewarm_s_chunk(0, mc, sbs0, mx40)
                       if mc < JC else None)
                nm0 = pwp.tile([128, 1], f32, tag="negmp0", name="negmp0")
                nc.vector.tensor_reduce(nm0, mx40, mybir.AxisListType.X,
                                        mybir.AluOpType.max, negate=True)
                prewarm_exp(0, sbs0, nm0)

            # ---- Phase 3: attention, software-pipelined over i-tiles
            # stage vt: S(vt) matmuls; DVE max + ACT exp follow on their
            # engines; PE then runs transposes(vt-2) and P@v(vt-3).
            with (
                tc.tile_pool(name="eti", bufs=3) as etip,
                tc.tile_pool(name="stg", bufs=2) as stgp,
                tc.tile_pool(name="sps", bufs=4, space="PSUM") as sps,
                tc.tile_pool(name="tps", bufs=2, space="PSUM") as tps,
                tc.tile_pool(name="ops", bufs=2, space="PSUM") as ops,
            ):
                e_ti = {}
                xr_t = {}

                s_ps = {}

                def stage_s_half(it, half):
                    itsl = slice(it * 128, (it + 1) * 128)
                    if half == 0:
                        mx4 = stgp.tile([128, 4], f32, tag="mx4",
                                        name=f"mx4_{it}")
                        s_ps[it] = ([], mx4)
                    ps_s, mx4 = s_ps[it]
                    for jc in (2 * half, 2 * half + 1):
                        jsl = slice(jc * 512, (jc + 1) * 512)
                        ps = sps.tile([128, 512], f32, tag="sps",
                                      name=f"sps{it}_{jc}")
                        nc.tensor.matmul(ps, qT0[:, itsl], kT0[:, jsl],
                                         start=True, stop=False)
                        nc.tensor.matmul(ps, qT1[:, itsl], kT1[:, jsl],
                                         start=False, stop=True)
                        # per-chunk max right away so exp can start early
                        nc.vector.tensor_reduce(
                            mx4[:, jc:jc + 1], ps, mybir.AxisListType.X,
                            mybir.AluOpType.max)
                        ps_s.append(ps)
                    if half == 1:
                        nm = stgp.tile([128, 1], f32, tag="negm",
                                       name=f"negm{it}")
                        nc.vector.tensor_reduce(nm, mx4, mybir.AxisListType.X,
                                                mybir.AluOpType.max,
                                                negate=True)
                        s_ps[it] = (ps_s, nm)

                def stage_exp(it):
                    # emitted one step after stage_s so the eti copies of the
                    # older i-tile go first in the ACT queue
                    ps_s, nm = s_ps.pop(it)
                    ebf = ebfp.tile([128, N], bf16, tag="ebf", name=f"ebf{it}")
                    for jc in range(JC):
                        jsl = slice(jc * 512, (jc + 1) * 512)
                        nc.scalar.activation(ebf[:, jsl], ps_s[jc],
                                             mybir.ActivationFunctionType.Exp,
                                             bias=nm)
                    e_bf[it] = ebf

                def stage_t_batch(it, b):
                    # 8 transposes fill one full psum bank; one wide ACT copy
                    # casts to fp8. Buffer reuse spans two blocks, so the PE
                    # never waits on the copy.
                    if b == 0:
                        ebf = e_bf.pop(it)
                        eti = etip.tile([128, NJ, 128], f8, tag="eti",
                                        name=f"eti{it}")
                        e_ti[it] = (eti, ebf)
                    eti, ebf = e_ti[it]
                    pt = tps.tile([128, 1024], bf16, tag="tps",
                                  name=f"tps{it}_{b}")
                    for k in range(8):
                        nc.tensor.transpose(
                            pt[:, k * 128:(k + 1) * 128],
                            ebf[:, (8 * b + k) * 128:(8 * b + k + 1) * 128],
                            id_t)
                    dst = eti[:, 8 * b:8 * b + 8, :]
                    src = pt.rearrange("p (b i) -> p b i", b=8)
                    nc.scalar.activation(
                        dst, src, mybir.ActivationFunctionType.Copy)

                # P@v m-chunks: chunk 0 carries the Z column (same accum
                # group — interleaved psum groups corrupt accumulation on hw)
                AV_CHUNKS = [(1120, 161, True), (0, 224, False),
                             (224, 224, False), (448, 224, False),
                             (672, 224, False), (896, 224, False)]

                def av_mc(it, mc, xr, osb, rz):
                    i0 = it * 128
                    eti = e_ti[it][0]
                    mlo, w, has_z = AV_CHUNKS[mc]
                    wv = w - 1 if has_z else w
                    po = ops.tile([128, 224], f32, tag="ops",
                                  name=f"ops{it}_{mc}")
                    for j in range(NJ):
                        lhs = eti[:, j, :].unsqueeze(1).broadcast_to(
                            [128, 2, 128])
                        nc.tensor.matmul(po[:, 0:w], lhs,
                                         v_t[j][:, :, mlo:mlo + w],
                                         start=(j == 0), stop=(j == NJ - 1),
                                         perf_mode=DR)
                    if has_z:
                        nc.vector.reciprocal(rz, po[:, wv:wv + 1])
                    nc.vector.scalar_tensor_tensor(
                        out=osb[:, mlo:mlo + wv], in0=po[:, 0:wv], scalar=rz,
                        in1=xr[:, mlo:mlo + wv], op0=mybir.AluOpType.mult,
                        op1=mybir.AluOpType.add)
                    if it == NIT - 1:
                        # drain: stream chunks out as they finish
                        nc.sync.dma_start(out=out_d[i0:i0 + 128, mlo:mlo + wv],
                                          in_=osb[:, mlo:mlo + wv])
                    elif mc == 5:
                        nc.sync.dma_start(out=out_d[i0:i0 + 128, :], in_=osb)

                def stage_av_pre(it):
                    if it + 1 < NIT:
                        xr = stgp.tile([128, L], f32, tag="xr",
                                       name=f"xr{it + 1}")
                        nc.sync.dma_start(
                            out=xr, in_=xr_d[(it + 1) * 128:(it + 2) * 128, :])
                        xr_t[it + 1] = xr
                    osb = stgp.tile([128, L], f32, tag="osb", name=f"osb{it}")
                    rz = stgp.tile([128, 1], f32, tag="rz", name=f"rz{it}")
                    return xr_t.pop(it), osb, rz

                xr0 = stgp.tile([128, L], f32, tag="xr", name="xr0")
                nc.sync.dma_start(out=xr0, in_=xr_d[0:128, :])
                xr_t[0] = xr0
                # PE order per block: transposes(vt-2) and P@v(vt-3) m-chunks
                # interleaved (psum-copy latencies hide behind matmuls), S(vt)
                # last; exp(vt-1) emitted after the transpose copies so the
                # ACT queue runs [eti copies, exps] with no head blocking.
                for vt in range(1, NIT + 3):
                    tr = 2 <= vt < NIT + 2
                    av = vt >= 3
                    if 2 <= vt < NIT + 1:
                        stage_exp(vt - 1)
                    if tr:
                        stage_t_batch(vt - 2, 0)
                    ctx = None
                    if av:
                        ctx = stage_av_pre(vt - 3)
                        av_mc(vt - 3, 0, *ctx)
                    if tr:
                        stage_t_batch(vt - 2, 1)
                    if av:
                        av_mc(vt - 3, 1, *ctx)
                    if vt < NIT:
                        stage_s_half(vt, 0)
                    if av:
                        av_mc(vt - 3, 2, *ctx)
                        av_mc(vt - 3, 3, *ctx)
                    if vt < NIT:
                        stage_s_half(vt, 1)
                    if av:
                        av_mc(vt - 3, 4, *ctx)
                        av_mc(vt - 3, 5, *ctx)
                        e_ti.pop(vt - 3)

            pwp.release()
            ebfp.release()

    nc.finalize()
    return nc


_NC = None


def _get_nc():
    global _NC
    if _NC is None:
        _NC = _build()
    return _NC


def kernel(x, Wq, bq, Wk, bk, Wv, bv):
    x = np.asarray(x, dtype=np.float32)
    WqT_full = np.asarray(Wq, np.float32).T                    # [L, H]
    WkT_full = np.asarray(Wk, np.float32).T                    # [L, H]
    def packw(a):  # [L, Hc] -> [128, NL, Hc]: partition-contiguous DMA layout
        return np.ascontiguousarray(
            a.reshape(NL, 128, a.shape[1]).transpose(1, 0, 2))

    WqT = packw(np.ascontiguousarray(WqT_full[:, :H0]))
    WkT = packw(np.ascontiguousarray(WkT_full[:, :H0]))
    Wqk1T = packw(np.ascontiguousarray(
        np.concatenate([WqT_full[:, H0:], WkT_full[:, H0:]], axis=1)))
    WvT = np.asarray(Wv, np.float32).T                         # [L, L]
    bq = np.asarray(bq, np.float32)
    bv = np.asarray(bv, np.float32)

    def pack8(a):  # [L, M] -> [128, NP, 2, M]
        return np.ascontiguousarray(
            a.reshape(NP, 2, 128, a.shape[1]).transpose(2, 0, 1, 3))

    def packw8(a):  # [128, NP, 2, L] -> [MC, 128, NP, 2, 256]
        return np.ascontiguousarray(
            a.reshape(128, NP, 2, MC, 256).transpose(3, 0, 1, 2, 4))

    Wv64 = (32.0 * WvT).astype(np.float32)
    Wh64f = Wv64.astype(E4)
    We64f = (Wv64 - Wh64f.astype(np.float32)).astype(E4)
    Wh64 = packw8(pack8(Wh64f))
    We64 = packw8(pack8(We64f))
    ident = np.eye(128, dtype=ml_dtypes.bfloat16)

    nc = _get_nc()
    in_maps = []
    for b in range(B):
        xT = np.ascontiguousarray(x[b].T)                      # [L, N]
        xh = xT.astype(E4)
        xe = (xT - xh.astype(np.float32)).astype(E4)
        in_maps.append({
            "xT": np.ascontiguousarray(
                xT.reshape(NL, 128, N).transpose(1, 0, 2)),
            "xh": pack8(xh),
            "xe": pack8(xe),
            "WqT": WqT,
            "WkT": WkT,
            "Wqk1T": Wqk1T,
            "Wh64": Wh64,
            "We64": We64,
            "bq": bq,
            "ident": ident,
            "xresid": x[b] + bv[None, :],
        })
    res = run_bass_kernel_spmd(nc, in_maps, list(range(B)))
    return np.stack([res.results[b]["out"] for b in range(B)], axis=0)


if __name__ == "__main__":
    rng = np.random.default_rng(0)
    ins = {
        "x": rng.standard_normal((B, N, L)).astype(np.float32),
        "Wq": rng.standard_normal((H, L)).astype(np.float32) * 0.028,
        "bq": rng.standard_normal((H,)).astype(np.float32) * 0.028,
        "Wk": rng.standard_normal((H, L)).astype(np.float32) * 0.028,
        "bk": rng.standard_normal((H,)).astype(np.float32) * 0.028,
        "Wv": rng.standard_normal((L, L)).astype(np.float32) * 0.028,
        "bv": rng.standard_normal((L,)).astype(np.float32) * 0.028,
    }
    out = kernel(**ins)
    print("kernel ran, out shape", out.shape)


# revision 48
# speedup vs baseline: 1.0492x; 1.0492x over previous
"""Bag self-attention kernel for TRN2, data-parallel over the bag dim (8 cores).

Per core (one bag, x: [N=2048, L=1280], H=160):
  q = x@Wq.T + bq ; k = x@Wk.T (bk cancels in softmax) ; v = x@Wv.T
  S = q@k.T ; P = softmax(S) ; out = P@v + (x + bv)      (gamma = 1)

Device-side strategy (fp8-DoubleRow accelerated):
  - q/k projections and S = q@k^T run in float32r (full PE rate at free>=256).
  - S is computed in [i, j] orientation so the per-row max is a free-dim
    reduce; exp(S - max) is an activation with a per-partition bias and
    writes E in bf16. E blocks are PE-transposed (bf16 identity) to [j, i]
    and cast to fp8e4 during the PSUM->SBUF copy.
  - v-projection runs as 3-term compensated fp8 DoubleRow at 32x scale:
    v32 = xh@Wh32 + xh@We32 + xe@Wh32 accumulated in one PSUM group, where
    xh/xe (fp8 value + fp8 error of x) and Wh32/We32 (fp8 of 32*WvT and its
    fp8 error) are prepared host-side; the We correction runs on only the
    first 2 of 5 l-pairs (error budget allows it). 1.2 rows per
    256-contraction instead of fp32r's 2.0.
  - v32 is split on-device into fp8 planes (vh, ve = v32 - vh), and the
    attention output is a DoubleRow matmul with planes (E, E) x (vh, ve),
    i.e. E@(vh+ve): v at ~16-bit effective precision, 2x the fp32r rate.
    The stationary E plane pair is a stride-0 broadcast AP (no duplicate).
  - The softmax normalizer rides the last P@v m-chunk via a constant
    column (32, 0) in the v planes: Z32 = sum_j E8 * 32, so the 32x scale
    on v cancels exactly in out = num * (1/Z32-col) and the normalization
    uses the same quantized E8 as the numerator (first-order quantization
    cancellation).
    The 32x scale keeps v32 under fp8e4's 240 max finite (the hw fp8e4 is
    IEEE e4m3 WITH infinities - overflow turns into inf, then NaN).
  - bv rides the residual (sum_j P == 1); bk drops (row-constant energy).
"""

import numpy as np
import ml_dtypes

import concourse.mybir as mybir
import concourse.tile as tile
from concourse import bacc
from concourse.bass_utils import run_bass_kernel_spmd

B, N, L, H = 8, 2048, 1280, 160
f32 = mybir.dt.float32
f32r = mybir.dt.float32r
bf16 = mybir.dt.bfloat16
f8 = mybir.dt.float8e4
DR = mybir.MatmulPerfMode.DoubleRow
E4 = ml_dtypes.float8_e4m3

NL = L // 128            # 10 l-chunks (f32r projection contraction)
NP = NL // 2             # 5 l-pairs (fp8 DoubleRow contraction)
NJ = N // 128            # 16 token blocks (j)
NIT = N // 128           # 16 i-tiles
JC = N // 512            # 4 j-chunks for S
MC = 5                   # m-chunks of 256
H0, H1 = 128, H - 128    # 128 + 32
PW = 2                   # l-pairs carrying the We correction term


def _build():
    nc = bacc.Bacc()
    xT_d = nc.declare_dram_parameter("xT", [128, NL, N], f32r, isOutput=False)
    xh_d = nc.declare_dram_parameter("xh", [128, NP, 2, N], f8, isOutput=False)
    xe_d = nc.declare_dram_parameter("xe", [128, NP, 2, N], f8, isOutput=False)
    # weight layouts pre-arranged host-side for contiguous per-partition DMA
    wq_d = nc.declare_dram_parameter("WqT", [128, NL, H0], f32r, isOutput=False)
    wk_d = nc.declare_dram_parameter("WkT", [128, NL, H0], f32r, isOutput=False)
    wqk1_d = nc.declare_dram_parameter("Wqk1T", [128, NL, 2 * H1], f32r,
                                       isOutput=False)
    wh_d = nc.declare_dram_parameter("Wh64", [MC, 128, NP, 2, 256], f8,
                                     isOutput=False)
    we_d = nc.declare_dram_parameter("We64", [MC, 128, NP, 2, 256], f8,
                                     isOutput=False)
    bq_d = nc.declare_dram_parameter("bq", [H], f32, isOutput=False)
    id_d = nc.declare_dram_parameter("ident", [128, 128], bf16, isOutput=False)
    xr_d = nc.declare_dram_parameter("xresid", [N, L], f32, isOutput=False)
    out_d = nc.declare_dram_parameter("out", [N, L], f32, isOutput=True)

    with tile.TileContext(nc) as tc:
        with (
            tc.tile_pool(name="const", bufs=1) as constp,
            tc.tile_pool(name="vpl", bufs=1) as vpool,
            tc.tile_pool(name="qkt", bufs=1) as qktp,
        ):
            bq_t = [constp.tile([H0, 1], f32, tag="bq0", name="bq0"),
                    constp.tile([H1, 1], f32, tag="bq1", name="bq1")]
            id_t = constp.tile([128, 128], bf16, tag="ident", name="ident")

            # v planes: [tok 128, (vh, ve), m 1280 + Z-col + pad] fp8 per token
            # block. Col L holds (32, 0) so the softmax normalizer Z*32 rides
            # the same accumulation group as the last P@v m-chunk.
            v_t = [vpool.tile([128, 2, L + 2], f8, tag=f"v{j}", name=f"v{j}")
                   for j in range(NJ)]

            # q/k resident: [h, N] f32r
            qT0 = qktp.tile([H0, N], f32r, tag="q0", name="q0")
            kT0 = qktp.tile([H0, N], f32r, tag="k0", name="k0")
            qT1 = qktp.tile([H1, N], f32r, tag="q1", name="q1")
            kT1 = qktp.tile([H1, N], f32r, tag="k1", name="k1")

            ebfp = tc.alloc_tile_pool(name="ebf", bufs=3)
            pwp = tc.alloc_tile_pool(name="pw", bufs=1)
            e_bf = {}

            # ---- Phase 1: projections, 4 token generations of 512
            with (
                tc.tile_pool(name="wqk", bufs=1) as wqkp,
                tc.tile_pool(name="w8", bufs=1) as w8p,
                tc.tile_pool(name="xt", bufs=2, side="right") as xtp,
                tc.tile_pool(name="x8", bufs=2, side="right") as x8p,
                tc.tile_pool(name="vps", bufs=4, space="PSUM") as vps,
                tc.tile_pool(name="qkps", bufs=1, space="PSUM") as qkps,
                tc.tile_pool(name="spre", bufs=1, space="PSUM") as sprep,
            ):
                def prewarm_s_chunk(it, jc, sbs, mx4):
                    # S + row-max chunk for an early i-tile, staged via SBUF
                    # so it needs only one psum bank alongside the P1 pools
                    itsl = slice(it * 128, (it + 1) * 128)
                    jsl = slice(jc * 512, (jc + 1) * 512)
                    ps = sprep.tile([128, 512], f32, tag="spre",
                                    name=f"spre{it}_{jc}")
                    nc.tensor.matmul(ps, qT0[:, itsl], kT0[:, jsl],
                                     start=True, stop=False)
                    nc.tensor.matmul(ps, qT1[:, itsl], kT1[:, jsl],
                                     start=False, stop=True)
                    nc.scalar.activation(sbs[:, jsl], ps,
                                         mybir.ActivationFunctionType.Copy)
                    nc.vector.tensor_reduce(
                        mx4[:, jc:jc + 1], ps, mybir.AxisListType.X,
                        mybir.AluOpType.max)

                def prewarm_exp(it, sbs, nm):
                    ebf = ebfp.tile([128, N], bf16, tag="ebf",
                                    name=f"ebf{it}")
                    for jc in range(JC):
                        jsl = slice(jc * 512, (jc + 1) * 512)
                        nc.scalar.activation(ebf[:, jsl], sbs[:, jsl],
                                             mybir.ActivationFunctionType.Exp,
                                             bias=nm)
                    e_bf[it] = ebf
                def dma_x_gen(g):
                    # one tile, two half-DMAs (overlaps other critical loads)
                    c0 = g * 512
                    t = xtp.tile([128, NL, 512], f32r, tag="x", name=f"x{g}")
                    nc.sync.dma_start(out=t[:, :, 0:256],
                                      in_=xT_d[:, :, c0:c0 + 256])
                    nc.sync.dma_start(out=t[:, :, 256:512],
                                      in_=xT_d[:, :, c0 + 256:c0 + 512])
                    return t

                def dma_x8_gen(g):
                    c0 = g * 512
                    th = x8p.tile([128, NP, 2, 512], f8, tag="xh",
                                  name=f"xh{g}")
                    nc.sync.dma_start(out=th, in_=xh_d[:, :, :, c0:c0 + 512])
                    te = x8p.tile([128, NP, 2, 512], f8, tag="xe",
                                  name=f"xe{g}")
                    nc.sync.dma_start(out=te, in_=xe_d[:, :, :, c0:c0 + 512])
                    return th, te

                def qk_proj(g, x_t):
                    isl = slice(g * 512, (g + 1) * 512)
                    ps3 = qkps.tile([2 * H1, 512], f32, tag="qk1ps",
                                    name=f"qk1ps{g}")
                    for l in range(NL):
                        nc.tensor.matmul(ps3, wqk1_t[:, l, :], x_t[:, l, :],
                                         start=(l == 0), stop=(l == NL - 1))
                    nc.vector.tensor_scalar_add(qT1[:, isl], ps3[0:H1, :], bq_t[1])
                    nc.any.tensor_copy(kT1[:, isl], ps3[H1:2 * H1, :])
                    ps = qkps.tile([H0, 512], f32, tag="qps", name=f"qps{g}")
                    for l in range(NL):
                        nc.tensor.matmul(ps, wq_t[:, l, :], x_t[:, l, :],
                                         start=(l == 0), stop=(l == NL - 1))
                    nc.vector.tensor_scalar_add(qT0[:, isl], ps, bq_t[0])
                    ps2 = qkps.tile([H0, 512], f32, tag="kps", name=f"kps{g}")
                    for l in range(NL):
                        nc.tensor.matmul(ps2, wk_t[:, l, :], x_t[:, l, :],
                                         start=(l == 0), stop=(l == NL - 1))
                    nc.any.tensor_copy(kT0[:, isl], ps2)

                def v_proj(g, xh_t, xe_t, hook=None):
                    for mc in range(MC):
                        if hook is not None:
                            hook(mc)
                        msl = slice(mc * 256, (mc + 1) * 256)
                        for t in range(4):
                            j = 4 * g + t
                            tsl = slice(t * 128, (t + 1) * 128)
                            ps = vps.tile([128, 256], f32, tag="vps",
                                          name=f"vps{g}_{mc}_{t}")
                            for p in range(NP):
                                nc.tensor.matmul(
                                    ps, xh_t[:, p, :, tsl], wh_t[:, mc, p],
                                    start=(p == 0), stop=False, perf_mode=DR)
                            for p in range(PW):
                                nc.tensor.matmul(
                                    ps, xh_t[:, p, :, tsl], we_t[:, mc, p],
                                    start=False, stop=False, perf_mode=DR)
                            for p in range(NP):
                                nc.tensor.matmul(
                                    ps, xe_t[:, p, :, tsl], wh_t[:, mc, p],
                                    start=False, stop=(p == NP - 1), perf_mode=DR)
                            # vh = fp8(v64); ve = fp8(v64 - vh)
                            if t % 2 == 0:
                                nc.vector.tensor_copy(v_t[j][:, 0, msl], ps)
                            else:
                                nc.scalar.activation(
                                    v_t[j][:, 0, msl], ps,
                                    mybir.ActivationFunctionType.Copy)
                            nc.vector.tensor_tensor(
                                out=v_t[j][:, 1, msl], in0=ps,
                                in1=v_t[j][:, 0, msl],
                                op=mybir.AluOpType.subtract)

                # DMA order: v_proj(0)'s operands first (fp8, small), then the
                # f32r q/k path loads while v_proj(0) runs on the PE.
                # DMA queue order tuned against per-chunk need-by times:
                # xe0 rides after the first weight chunk (the xe term is the
                # last of the three in each psum group)
                th0 = x8p.tile([128, NP, 2, 512], f8, tag="xh", name="xh0")
                nc.sync.dma_start(out=th0, in_=xh_d[:, :, :, 0:512])
                wh_t = w8p.tile([128, MC, NP, 2, 256], f8, tag="wh", name="wh")
                we_t = w8p.tile([128, MC, PW, 2, 256], f8, tag="we", name="we")

                def dma_w8(mc):
                    nc.sync.dma_start(out=wh_t[:, mc], in_=wh_d[mc])
                    nc.sync.dma_start(out=we_t[:, mc], in_=we_d[mc][:, 0:PW])

                dma_w8(0)
                te0 = x8p.tile([128, NP, 2, 512], f8, tag="xe", name="xe0")
                nc.sync.dma_start(out=te0, in_=xe_d[:, :, :, 0:512])
                xh_cur = (th0, te0)
                dma_w8(1)
                wqk1_t = wqkp.tile([128, NL, 2 * H1], f32r, tag="wqk1",
                                   name="wqk1")
                nc.sync.dma_start(out=wqk1_t, in_=wqk1_d[:, :, :])
                x_cur = xtp.tile([128, NL, 512], f32r, tag="x", name="x0")
                nc.sync.dma_start(out=x_cur[:, :, 0:256], in_=xT_d[:, :, 0:256])
                dma_w8(2)
                dma_w8(3)
                dma_w8(4)
                xh1t = x8p.tile([128, NP, 2, 512], f8, tag="xh", name="xh1")
                nc.sync.dma_start(out=xh1t, in_=xh_d[:, :, :, 512:1024])
                nc.sync.dma_start(out=x_cur[:, :, 256:512],
                                  in_=xT_d[:, :, 256:512])
                xe1t = x8p.tile([128, NP, 2, 512], f8, tag="xe", name="xe1")
                nc.sync.dma_start(out=xe1t, in_=xe_d[:, :, :, 512:1024])
                xh_next = (xh1t, xe1t)
                wq_t = wqkp.tile([128, NL, H0], f32r, tag="wq", name="wq")
                nc.sync.dma_start(out=wq_t, in_=wq_d[:, :, :])
                wk_t = wqkp.tile([128, NL, H0], f32r, tag="wk", name="wk")
                nc.sync.dma_start(out=wk_t, in_=wk_d[:, :, :])
                nc.sync.dma_start(out=bq_t[0], in_=bq_d[0:H0].unsqueeze(1))
                nc.sync.dma_start(out=bq_t[1], in_=bq_d[H0:H].unsqueeze(1))
                nc.sync.dma_start(out=id_t, in_=id_d[:, :])
                for j in range(NJ):
                    nc.vector.memset(v_t[j][:, :, L:L + 2], 0.0)
                    nc.vector.memset(v_t[j][:, 0, L:L + 1], 32.0)

                v_proj(0, *xh_cur)
                x_next = dma_x_gen(1)
                qk_proj(0, x_cur)
                for g in (1, 2):
                    x_cur, xh_cur = x_next, xh_next
                    xh_next = dma_x8_gen(g + 1)
                    v_proj(g, *xh_cur)
                    x_next = dma_x_gen(g + 1)
                    qk_proj(g, x_cur)
                # gen 3: projections first, then pre-warm the first two
                # attention i-tiles while v_proj(3) keeps the PE busy
                qk_proj(3, x_next)
                sbs0 = pwp.tile([128, N], f32, tag="sbs0", name="sbs0")
                mx40 = pwp.tile([128, 4], f32, tag="mx4p0", name="mx4p0")
                v_proj(3, *xh_next,
                       hook=lambda mc: prewarm_s_chunk(0, mc, sbs0, mx40)
                       if mc < JC else None)
                nm0 = pwp.tile([128, 1], f32, tag="negmp0", name="negmp0")
                nc.vector.tensor_reduce(nm0, mx40, mybir.AxisListType.X,
                                        mybir.AluOpType.max, negate=True)
                prewarm_exp(0, sbs0, nm0)

            # ---- Phase 3: attention, software-pipelined over i-tiles
            # stage vt: S(vt) matmuls; DVE max + ACT exp follow on their
            # engines; PE then runs transposes(vt-2) and P@v(vt-3).
            with (
                tc.tile_pool(name="eti", bufs=3) as etip,
                tc.tile_pool(name="stg", bufs=2) as stgp,
                tc.tile_pool(name="sps", bufs=4, space="PSUM") as sps,
                tc.tile_pool(name="tps", bufs=2, space="PSUM") as tps,
                tc.tile_pool(name="ops", bufs=2, space="PSUM") as ops,
            ):
                e_ti = {}
                xr_t = {}

                s_ps = {}

                def stage_s_half(it, half):
                    itsl = slice(it * 128, (it + 1) * 128)
                    if half == 0:
                        mx4 = stgp.tile([128, 4], f32, tag="mx4",
                                        name=f"mx4_{it}")
                        s_ps[it] = ([], mx4)
                    ps_s, mx4 = s_ps[it]
                    for jc in (2 * half, 2 * half + 1):
                        jsl = slice(jc * 512, (jc + 1) * 512)
                        ps = sps.tile([128, 512], f32, tag="sps",
                                      name=f"sps{it}_{jc}")
                        nc.tensor.matmul(ps, qT0[:, itsl], kT0[:, jsl],
                                         start=True, stop=False)
                        nc.tensor.matmul(ps, qT1[:, itsl], kT1[:, jsl],
                                         start=False, stop=True)
                        # per-chunk max right away so exp can start early
                        nc.vector.tensor_reduce(
                            mx4[:, jc:jc + 1], ps, mybir.AxisListType.X,
                            mybir.AluOpType.max)
                        ps_s.append(ps)
                    if half == 1:
                        nm = stgp.tile([128, 1], f32, tag="negm",
                                       name=f"negm{it}")
                        nc.vector.tensor_reduce(nm, mx4, mybir.AxisListType.X,
                                                mybir.AluOpType.max,
                                                negate=True)
                        s_ps[it] = (ps_s, nm)

                def stage_exp(it):
                    # emitted one step after stage_s so the eti copies of the
                    # older i-tile go first in the ACT queue
                    ps_s, nm = s_ps.pop(it)
                    ebf = ebfp.tile([128, N], bf16, tag="ebf", name=f"ebf{it}")
                    for jc in range(JC):
                        jsl = slice(jc * 512, (jc + 1) * 512)
                        nc.scalar.activation(ebf[:, jsl], ps_s[jc],
                                             mybir.ActivationFunctionType.Exp,
                                             bias=nm)
                    e_bf[it] = ebf

                def stage_t_batch(it, b):
                    # 8 transposes fill one full psum bank; one wide ACT copy
                    # casts to fp8. Buffer reuse spans two blocks, so the PE
                    # never waits on the copy.
                    if b == 0:
                        ebf = e_bf.pop(it)
                        eti = etip.tile([128, NJ, 128], f8, tag="eti",
                                        name=f"eti{it}")
                        e_ti[it] = (eti, ebf)
                    eti, ebf = e_ti[it]
                    pt = tps.tile([128, 1024], bf16, tag="tps",
                                  name=f"tps{it}_{b}")
                    for k in range(8):
                        nc.tensor.transpose(
                            pt[:, k * 128:(k + 1) * 128],
                            ebf[:, (8 * b + k) * 128:(8 * b + k + 1) * 128],
                            id_t)
                    dst = eti[:, 8 * b:8 * b + 8, :]
                    src = pt.rearrange("p (b i) -> p b i", b=8)
                    nc.scalar.activation(
                        dst, src, mybir.ActivationFunctionType.Copy)

                # P@v m-chunks: chunk 0 carries the Z column (same accum
                # group — interleaved psum groups corrupt accumulation on hw)
                AV_CHUNKS = [(1120, 161, True), (0, 224, False),
                             (224, 224, False), (448, 224, False),
                             (672, 224, False), (896, 224, False)]

                def av_mc(it, mc, xr, osb, rz):
                    i0 = it * 128
                    eti = e_ti[it][0]
                    mlo, w, has_z = AV_CHUNKS[mc]
                    wv = w - 1 if has_z else w
                    po = ops.tile([128, 224], f32, tag="ops",
                                  name=f"ops{it}_{mc}")
                    for j in range(NJ):
                        lhs = eti[:, j, :].unsqueeze(1).broadcast_to(
                            [128, 2, 128])
                        nc.tensor.matmul(po[:, 0:w], lhs,
                                         v_t[j][:, :, mlo:mlo + w],
                                         start=(j == 0), stop=(j == NJ - 1),
                                         perf_mode=DR)
                    if has_z:
                        nc.vector.reciprocal(rz, po[:, wv:wv + 1])
                    nc.vector.scalar_tensor_tensor(
                        out=osb[:, mlo:mlo + wv], in0=po[:, 0:wv], scalar=rz,
                        in1=xr[:, mlo:mlo + wv], op0=mybir.AluOpType.mult,
                        op1=mybir.AluOpType.add)
                    if it == NIT - 1:
                        # drain: stream chunks out as they finish
                        nc.sync.dma_start(out=out_d[i0:i0 + 128, mlo:mlo + wv],
                                          in_=osb[:, mlo:mlo + wv])
                    elif mc == 5:
                        nc.sync.dma_start(out=out_d[i0:i0 + 128, :], in_=osb)

                def stage_av_pre(it):
                    if it + 1 < NIT:
                        xr = stgp.tile([128, L], f32, tag="xr",
                                       name=f"xr{it + 1}")
                        nc.sync.dma_start(
                            out=xr, in_=xr_d[(it + 1) * 128:(it + 2) * 128, :])
                        xr_t[it + 1] = xr
                    osb = stgp.tile([128, L], f32, tag="osb", name=f"osb{it}")
                    rz = stgp.tile([128, 1], f32, tag="rz", name=f"rz{it}")
                    return xr_t.pop(it), osb, rz

                xr0 = stgp.tile([128, L], f32, tag="xr", name="xr0")
                nc.sync.dma_start(out=xr0, in_=xr_d[0:128, :])
                xr_t[0] = xr0
                # PE order per block: transposes(vt-2) and P@v(vt-3) m-chunks
                # interleaved (psum-copy latencies hide behind matmuls), S(vt)
                # last; exp(vt-1) emitted after the transpose copies so the
                # ACT queue runs [eti copies, exps] with no head blocking.
                for vt in range(1, NIT + 3):
                    tr = 2 <= vt < NIT + 2
                    av = vt >= 3
                    if 2 <= vt < NIT + 1:
                        stage_exp(vt - 1)
                    if tr:
                        stage_t_batch(vt - 2, 0)
                    ctx = None
                    if av:
                        ctx = stage_av_pre(vt - 3)
                        av_mc(vt - 3, 0, *ctx)
                    if tr:
                        stage_t_batch(vt - 2, 1)
                    if av:
                        av_mc(vt - 3, 1, *ctx)
                    if vt < NIT:
                        stage_s_half(vt, 0)
                    if av:
                        av_mc(vt - 3, 2, *ctx)
                        av_mc(vt - 3, 3, *ctx)
                    if vt < NIT:
                        stage_s_half(vt, 1)
                    if av:
                        av_mc(vt - 3, 4, *ctx)
                        av_mc(vt - 3, 5, *ctx)
                        e_ti.pop(vt - 3)

            pwp.release()
            ebfp.release()

    nc.finalize()
    return nc


_NC = None


def _get_nc():
    global _NC
    if _NC is None:
        _NC = _build()
    return _NC


def kernel(x, Wq, bq, Wk, bk, Wv, bv):
    x = np.asarray(x, dtype=np.float32)
    WqT_full = np.asarray(Wq, np.float32).T                    # [L, H]
    WkT_full = np.asarray(Wk, np.float32).T                    # [L, H]
    def packw(a):  # [L, Hc] -> [128, NL, Hc]: partition-contiguous DMA layout
        return np.ascontiguousarray(
            a.reshape(NL, 128, a.shape[1]).transpose(1, 0, 2))

    WqT = packw(np.ascontiguousarray(WqT_full[:, :H0]))
    WkT = packw(np.ascontiguousarray(WkT_full[:, :H0]))
    Wqk1T = packw(np.ascontiguousarray(
        np.concatenate([WqT_full[:, H0:], WkT_full[:, H0:]], axis=1)))
    WvT = np.asarray(Wv, np.float32).T                         # [L, L]
    bq = np.asarray(bq, np.float32)
    bv = np.asarray(bv, np.float32)

    def pack8(a):  # [L, M] -> [128, NP, 2, M]
        return np.ascontiguousarray(
            a.reshape(NP, 2, 128, a.shape[1]).transpose(2, 0, 1, 3))

    def packw8(a):  # [128, NP, 2, L] -> [MC, 128, NP, 2, 256]
        return np.ascontiguousarray(
            a.reshape(128, NP, 2, MC, 256).transpose(3, 0, 1, 2, 4))

    Wv64 = (32.0 * WvT).astype(np.float32)
    Wh64f = Wv64.astype(E4)
    We64f = (Wv64 - Wh64f.astype(np.float32)).astype(E4)
    Wh64 = packw8(pack8(Wh64f))
    We64 = packw8(pack8(We64f))
    ident = np.eye(128, dtype=ml_dtypes.bfloat16)

    nc = _get_nc()
    in_maps = []
    for b in range(B):
        xT = np.ascontiguousarray(x[b].T)                      # [L, N]
        xh = xT.astype(E4)
        xe = (xT - xh.astype(np.float32)).astype(E4)
        in_maps.append({
            "xT": np.ascontiguousarray(
                xT.reshape(NL, 128, N).transpose(1, 0, 2)),
            "xh": pack8(xh),
            "xe": pack8(xe),
            "WqT": WqT,
            "WkT": WkT,
            "Wqk1T": Wqk1T,
            "Wh64": Wh64,
            "We64": We64,
            "bq": bq,
            "ident": ident,
            "xresid": x[b] + bv[None, :],
        })
    res = run_bass_kernel_spmd(nc, in_maps, list(range(B)))
    return np.stack([res.results[b]["out"] for b in range(B)], axis=0)


if __name__ == "__main__":
    rng = np.random.default_rng(0)
    ins = {
        "x": rng.standard_normal((B, N, L)).astype(np.float32),
        "Wq": rng.standard_normal((H, L)).astype(np.float32) * 0.028,
        "bq": rng.standard_normal((H,)).astype(np.float32) * 0.028,
        "Wk": rng.standard_normal((H, L)).astype(np.float32) * 0.028,
        "bk": rng.standard_normal((H,)).astype(np.float32) * 0.028,
        "Wv": rng.standard_normal((L, L)).astype(np.float32) * 0.028,
        "bv": rng.standard_normal((L,)).astype(np.float32) * 0.028,
    }
    out = kernel(**ins)
    print("kernel ran, out shape", out.shape)


# revision 50
# speedup vs baseline: 1.0583x; 1.0087x over previous
"""Bag self-attention kernel for TRN2, data-parallel over the bag dim (8 cores).

Per core (one bag, x: [N=2048, L=1280], H=160):
  q = x@Wq.T + bq ; k = x@Wk.T (bk cancels in softmax) ; v = x@Wv.T
  S = q@k.T ; P = softmax(S) ; out = P@v + (x + bv)      (gamma = 1)

Device-side strategy (fp8-DoubleRow accelerated):
  - q/k projections and S = q@k^T run in float32r (full PE rate at free>=256).
  - S is computed in [i, j] orientation so the per-row max is a free-dim
    reduce; exp(S - max) is an activation with a per-partition bias and
    writes E in bf16. E blocks are PE-transposed (bf16 identity) to [j, i]
    and cast to fp8e4 during the PSUM->SBUF copy.
  - v-projection runs as 3-term compensated fp8 DoubleRow at 32x scale:
    v32 = xh@Wh32 + xh@We32 + xe@Wh32 accumulated in one PSUM group, where
    xh/xe (fp8 value + fp8 error of x) and Wh32/We32 (fp8 of 32*WvT and its
    fp8 error) are prepared host-side; the We correction runs on only 2 and
    the xe correction on 4 of the 5 l-pairs (error budget allows it).
    1.1 rows per 256-contraction instead of fp32r's 2.0.
  - v32 is split on-device into fp8 planes (vh, ve = v32 - vh), and the
    attention output is a DoubleRow matmul with planes (E, E) x (vh, ve),
    i.e. E@(vh+ve): v at ~16-bit effective precision, 2x the fp32r rate.
    The stationary E plane pair is a stride-0 broadcast AP (no duplicate).
  - The softmax normalizer rides the last P@v m-chunk via a constant
    column (32, 0) in the v planes: Z32 = sum_j E8 * 32, so the 32x scale
    on v cancels exactly in out = num * (1/Z32-col) and the normalization
    uses the same quantized E8 as the numerator (first-order quantization
    cancellation).
    The 32x scale keeps v32 under fp8e4's 240 max finite (the hw fp8e4 is
    IEEE e4m3 WITH infinities - overflow turns into inf, then NaN).
  - bv rides the residual (sum_j P == 1); bk drops (row-constant energy).
"""

import numpy as np
import ml_dtypes

import concourse.mybir as mybir
import concourse.tile as tile
from concourse import bacc
from concourse.bass_utils import run_bass_kernel_spmd

B, N, L, H = 8, 2048, 1280, 160
f32 = mybir.dt.float32
f32r = mybir.dt.float32r
bf16 = mybir.dt.bfloat16
f8 = mybir.dt.float8e4
DR = mybir.MatmulPerfMode.DoubleRow
E4 = ml_dtypes.float8_e4m3

NL = L // 128            # 10 l-chunks (f32r projection contraction)
NP = NL // 2             # 5 l-pairs (fp8 DoubleRow contraction)
NJ = N // 128            # 16 token blocks (j)
NIT = N // 128           # 16 i-tiles
JC = N // 512            # 4 j-chunks for S
MC = 5                   # m-chunks of 256
H0, H1 = 128, H - 128    # 128 + 32
PW = 2                   # l-pairs carrying the We correction term
PX = 4                   # l-pairs carrying the xe correction term


def _build():
    nc = bacc.Bacc()
    xT_d = nc.declare_dram_parameter("xT", [128, NL, N], f32r, isOutput=False)
    xh_d = nc.declare_dram_parameter("xh", [128, NP, 2, N], f8, isOutput=False)
    xe_d = nc.declare_dram_parameter("xe", [128, NP, 2, N], f8, isOutput=False)
    # weight layouts pre-arranged host-side for contiguous per-partition DMA
    wq_d = nc.declare_dram_parameter("WqT", [128, NL, H0], f32r, isOutput=False)
    wk_d = nc.declare_dram_parameter("WkT", [128, NL, H0], f32r, isOutput=False)
    wqk1_d = nc.declare_dram_parameter("Wqk1T", [128, NL, 2 * H1], f32r,
                                       isOutput=False)
    wh_d = nc.declare_dram_parameter("Wh64", [MC, 128, NP, 2, 256], f8,
                                     isOutput=False)
    we_d = nc.declare_dram_parameter("We64", [MC, 128, NP, 2, 256], f8,
                                     isOutput=False)
    bq_d = nc.declare_dram_parameter("bq", [H], f32, isOutput=False)
    id_d = nc.declare_dram_parameter("ident", [128, 128], bf16, isOutput=False)
    xr_d = nc.declare_dram_parameter("xresid", [N, L], f32, isOutput=False)
    out_d = nc.declare_dram_parameter("out", [N, L], f32, isOutput=True)

    with tile.TileContext(nc) as tc:
        with (
            tc.tile_pool(name="const", bufs=1) as constp,
            tc.tile_pool(name="vpl", bufs=1) as vpool,
            tc.tile_pool(name="qkt", bufs=1) as qktp,
        ):
            bq_t = [constp.tile([H0, 1], f32, tag="bq0", name="bq0"),
                    constp.tile([H1, 1], f32, tag="bq1", name="bq1")]
            id_t = constp.tile([128, 128], bf16, tag="ident", name="ident")

            # v planes: [tok 128, (vh, ve), m 1280 + Z-col + pad] fp8 per token
            # block. Col L holds (32, 0) so the softmax normalizer Z*32 rides
            # the same accumulation group as the last P@v m-chunk.
            v_t = [vpool.tile([128, 2, L + 2], f8, tag=f"v{j}", name=f"v{j}")
                   for j in range(NJ)]

            # q/k resident: [h, N] f32r
            qT0 = qktp.tile([H0, N], f32r, tag="q0", name="q0")
            kT0 = qktp.tile([H0, N], f32r, tag="k0", name="k0")
            qT1 = qktp.tile([H1, N], f32r, tag="q1", name="q1")
            kT1 = qktp.tile([H1, N], f32r, tag="k1", name="k1")

            ebfp = tc.alloc_tile_pool(name="ebf", bufs=3)
            pwp = tc.alloc_tile_pool(name="pw", bufs=1)
            e_bf = {}

            # ---- Phase 1: projections, 4 token generations of 512
            with (
                tc.tile_pool(name="wqk", bufs=1) as wqkp,
                tc.tile_pool(name="w8", bufs=1) as w8p,
                tc.tile_pool(name="xt", bufs=2, side="right") as xtp,
                tc.tile_pool(name="x8", bufs=2, side="right") as x8p,
                tc.tile_pool(name="vps", bufs=4, space="PSUM") as vps,
                tc.tile_pool(name="qkps", bufs=1, space="PSUM") as qkps,
                tc.tile_pool(name="spre", bufs=1, space="PSUM") as sprep,
            ):
                def prewarm_s_chunk(it, jc, sbs, mx4):
                    # S + row-max chunk for an early i-tile, staged via SBUF
                    # so it needs only one psum bank alongside the P1 pools
                    itsl = slice(it * 128, (it + 1) * 128)
                    jsl = slice(jc * 512, (jc + 1) * 512)
                    ps = sprep.tile([128, 512], f32, tag="spre",
                                    name=f"spre{it}_{jc}")
                    nc.tensor.matmul(ps, qT0[:, itsl], kT0[:, jsl],
                                     start=True, stop=False)
                    nc.tensor.matmul(ps, qT1[:, itsl], kT1[:, jsl],
                                     start=False, stop=True)
                    nc.scalar.activation(sbs[:, jsl], ps,
                                         mybir.ActivationFunctionType.Copy)
                    nc.vector.tensor_reduce(
                        mx4[:, jc:jc + 1], ps, mybir.AxisListType.X,
                        mybir.AluOpType.max)

                def prewarm_exp(it, sbs, nm):
                    ebf = ebfp.tile([128, N], bf16, tag="ebf",
                                    name=f"ebf{it}")
                    for jc in range(JC):
                        jsl = slice(jc * 512, (jc + 1) * 512)
                        nc.scalar.activation(ebf[:, jsl], sbs[:, jsl],
                                             mybir.ActivationFunctionType.Exp,
                                             bias=nm)
                    e_bf[it] = ebf
                def dma_x_gen(g):
                    # one tile, two half-DMAs (overlaps other critical loads)
                    c0 = g * 512
                    t = xtp.tile([128, NL, 512], f32r, tag="x", name=f"x{g}")
                    nc.sync.dma_start(out=t[:, :, 0:256],
                                      in_=xT_d[:, :, c0:c0 + 256])
                    nc.sync.dma_start(out=t[:, :, 256:512],
                                      in_=xT_d[:, :, c0 + 256:c0 + 512])
                    return t

                def dma_x8_gen(g):
                    c0 = g * 512
                    th = x8p.tile([128, NP, 2, 512], f8, tag="xh",
                                  name=f"xh{g}")
                    nc.sync.dma_start(out=th, in_=xh_d[:, :, :, c0:c0 + 512])
                    te = x8p.tile([128, PX, 2, 512], f8, tag="xe",
                                  name=f"xe{g}")
                    nc.sync.dma_start(out=te,
                                      in_=xe_d[:, 0:PX, :, c0:c0 + 512])
                    return th, te

                def qk_proj(g, x_t):
                    isl = slice(g * 512, (g + 1) * 512)
                    ps3 = qkps.tile([2 * H1, 512], f32, tag="qk1ps",
                                    name=f"qk1ps{g}")
                    for l in range(NL):
                        nc.tensor.matmul(ps3, wqk1_t[:, l, :], x_t[:, l, :],
                                         start=(l == 0), stop=(l == NL - 1))
                    nc.vector.tensor_scalar_add(qT1[:, isl], ps3[0:H1, :], bq_t[1])
                    nc.any.tensor_copy(kT1[:, isl], ps3[H1:2 * H1, :])
                    ps = qkps.tile([H0, 512], f32, tag="qps", name=f"qps{g}")
                    for l in range(NL):
                        nc.tensor.matmul(ps, wq_t[:, l, :], x_t[:, l, :],
                                         start=(l == 0), stop=(l == NL - 1))
                    nc.vector.tensor_scalar_add(qT0[:, isl], ps, bq_t[0])
                    ps2 = qkps.tile([H0, 512], f32, tag="kps", name=f"kps{g}")
                    for l in range(NL):
                        nc.tensor.matmul(ps2, wk_t[:, l, :], x_t[:, l, :],
                                         start=(l == 0), stop=(l == NL - 1))
                    nc.any.tensor_copy(kT0[:, isl], ps2)

                def v_proj(g, xh_t, xe_t, hook=None):
                    for mc in range(MC):
                        if hook is not None:
                            hook(mc)
                        msl = slice(mc * 256, (mc + 1) * 256)
                        for t in range(4):
                            j = 4 * g + t
                            tsl = slice(t * 128, (t + 1) * 128)
                            ps = vps.tile([128, 256], f32, tag="vps",
                                          name=f"vps{g}_{mc}_{t}")
                            for p in range(NP):
                                nc.tensor.matmul(
                                    ps, xh_t[:, p, :, tsl], wh_t[:, mc, p],
                                    start=(p == 0), stop=False, perf_mode=DR)
                            for p in range(PW):
                                nc.tensor.matmul(
                                    ps, xh_t[:, p, :, tsl], we_t[:, mc, p],
                                    start=False, stop=False, perf_mode=DR)
                            for p in range(PX):
                                nc.tensor.matmul(
                                    ps, xe_t[:, p, :, tsl], wh_t[:, mc, p],
                                    start=False, stop=(p == PX - 1), perf_mode=DR)
                            # vh = fp8(v64); ve = fp8(v64 - vh)
                            if t % 2 == 0:
                                nc.vector.tensor_copy(v_t[j][:, 0, msl], ps)
                            else:
                                nc.scalar.activation(
                                    v_t[j][:, 0, msl], ps,
                                    mybir.ActivationFunctionType.Copy)
                            nc.vector.tensor_tensor(
                                out=v_t[j][:, 1, msl], in0=ps,
                                in1=v_t[j][:, 0, msl],
                                op=mybir.AluOpType.subtract)

                # DMA order: v_proj(0)'s operands first (fp8, small), then the
                # f32r q/k path loads while v_proj(0) runs on the PE.
                # DMA queue order tuned against per-chunk need-by times:
                # xe0 rides after the first weight chunk (the xe term is the
                # last of the three in each psum group)
                th0 = x8p.tile([128, NP, 2, 512], f8, tag="xh", name="xh0")
                nc.sync.dma_start(out=th0, in_=xh_d[:, :, :, 0:512])
                wh_t = w8p.tile([128, MC, NP, 2, 256], f8, tag="wh", name="wh")
                we_t = w8p.tile([128, MC, PW, 2, 256], f8, tag="we", name="we")

                def dma_w8(mc):
                    nc.sync.dma_start(out=wh_t[:, mc], in_=wh_d[mc])
                    nc.sync.dma_start(out=we_t[:, mc], in_=we_d[mc][:, 0:PW])

                dma_w8(0)
                te0 = x8p.tile([128, PX, 2, 512], f8, tag="xe", name="xe0")
                nc.sync.dma_start(out=te0, in_=xe_d[:, 0:PX, :, 0:512])
                xh_cur = (th0, te0)
                dma_w8(1)
                wqk1_t = wqkp.tile([128, NL, 2 * H1], f32r, tag="wqk1",
                                   name="wqk1")
                nc.sync.dma_start(out=wqk1_t, in_=wqk1_d[:, :, :])
                x_cur = xtp.tile([128, NL, 512], f32r, tag="x", name="x0")
                nc.sync.dma_start(out=x_cur[:, :, 0:256], in_=xT_d[:, :, 0:256])
                dma_w8(2)
                dma_w8(3)
                dma_w8(4)
                xh1t = x8p.tile([128, NP, 2, 512], f8, tag="xh", name="xh1")
                nc.sync.dma_start(out=xh1t, in_=xh_d[:, :, :, 512:1024])
                nc.sync.dma_start(out=x_cur[:, :, 256:512],
                                  in_=xT_d[:, :, 256:512])
                xe1t = x8p.tile([128, PX, 2, 512], f8, tag="xe", name="xe1")
                nc.sync.dma_start(out=xe1t, in_=xe_d[:, 0:PX, :, 512:1024])
                xh_next = (xh1t, xe1t)
                wq_t = wqkp.tile([128, NL, H0], f32r, tag="wq", name="wq")
                nc.sync.dma_start(out=wq_t, in_=wq_d[:, :, :])
                wk_t = wqkp.tile([128, NL, H0], f32r, tag="wk", name="wk")
                nc.sync.dma_start(out=wk_t, in_=wk_d[:, :, :])
                nc.sync.dma_start(out=bq_t[0], in_=bq_d[0:H0].unsqueeze(1))
                nc.sync.dma_start(out=bq_t[1], in_=bq_d[H0:H].unsqueeze(1))
                nc.sync.dma_start(out=id_t, in_=id_d[:, :])
                for j in range(NJ):
                    nc.vector.memset(v_t[j][:, :, L:L + 2], 0.0)
                    nc.vector.memset(v_t[j][:, 0, L:L + 1], 32.0)

                v_proj(0, *xh_cur)
                x_next = dma_x_gen(1)
                qk_proj(0, x_cur)
                for g in (1, 2):
                    x_cur, xh_cur = x_next, xh_next
                    xh_next = dma_x8_gen(g + 1)
                    v_proj(g, *xh_cur)
                    x_next = dma_x_gen(g + 1)
                    qk_proj(g, x_cur)
                # gen 3: projections first, then pre-warm the first two
                # attention i-tiles while v_proj(3) keeps the PE busy
                qk_proj(3, x_next)
                sbs0 = pwp.tile([128, N], f32, tag="sbs0", name="sbs0")
                mx40 = pwp.tile([128, 4], f32, tag="mx4p0", name="mx4p0")
                v_proj(3, *xh_next,
                       hook=lambda mc: prewarm_s_chunk(0, mc, sbs0, mx40)
                       if mc < JC else None)
                nm0 = pwp.tile([128, 1], f32, tag="negmp0", name="negmp0")
                nc.vector.tensor_reduce(nm0, mx40, mybir.AxisListType.X,
                                        mybir.AluOpType.max, negate=True)
                prewarm_exp(0, sbs0, nm0)

            # ---- Phase 3: attention, software-pipelined over i-tiles
            # stage vt: S(vt) matmuls; DVE max + ACT exp follow on their
            # engines; PE then runs transposes(vt-2) and P@v(vt-3).
            with (
                tc.tile_pool(name="eti", bufs=3) as etip,
                tc.tile_pool(name="stg", bufs=2) as stgp,
                tc.tile_pool(name="sps", bufs=4, space="PSUM") as sps,
                tc.tile_pool(name="tps", bufs=2, space="PSUM") as tps,
                tc.tile_pool(name="ops", bufs=2, space="PSUM") as ops,
            ):
                e_ti = {}
                xr_t = {}

                s_ps = {}

                def stage_s_half(it, half):
                    itsl = slice(it * 128, (it + 1) * 128)
                    if half == 0:
                        mx4 = stgp.tile([128, 4], f32, tag="mx4",
                                        name=f"mx4_{it}")
                        s_ps[it] = ([], mx4)
                    ps_s, mx4 = s_ps[it]
                    for jc in (2 * half, 2 * half + 1):
                        jsl = slice(jc * 512, (jc + 1) * 512)
                        ps = sps.tile([128, 512], f32, tag="sps",
                                      name=f"sps{it}_{jc}")
                        nc.tensor.matmul(ps, qT0[:, itsl], kT0[:, jsl],
                                         start=True, stop=False)
                        nc.tensor.matmul(ps, qT1[:, itsl], kT1[:, jsl],
                                         start=False, stop=True)
                        # per-chunk max right away so exp can start early
                        nc.vector.tensor_reduce(
                            mx4[:, jc:jc + 1], ps, mybir.AxisListType.X,
                            mybir.AluOpType.max)
                        ps_s.append(ps)
                    if half == 1:
                        nm = stgp.tile([128, 1], f32, tag="negm",
                                       name=f"negm{it}")
                        nc.vector.tensor_reduce(nm, mx4, mybir.AxisListType.X,
                                                mybir.AluOpType.max,
                                                negate=True)
                        s_ps[it] = (ps_s, nm)

                def stage_exp(it):
                    # emitted one step after stage_s so the eti copies of the
                    # older i-tile go first in the ACT queue
                    ps_s, nm = s_ps.pop(it)
                    ebf = ebfp.tile([128, N], bf16, tag="ebf", name=f"ebf{it}")
                    for jc in range(JC):
                        jsl = slice(jc * 512, (jc + 1) * 512)
                        nc.scalar.activation(ebf[:, jsl], ps_s[jc],
                                             mybir.ActivationFunctionType.Exp,
                                             bias=nm)
                    e_bf[it] = ebf

                def stage_t_batch(it, b):
                    # 8 transposes fill one full psum bank; one wide ACT copy
                    # casts to fp8. Buffer reuse spans two blocks, so the PE
                    # never waits on the copy.
                    if b == 0:
                        ebf = e_bf.pop(it)
                        eti = etip.tile([128, NJ, 128], f8, tag="eti",
                                        name=f"eti{it}")
                        e_ti[it] = (eti, ebf)
                    eti, ebf = e_ti[it]
                    pt = tps.tile([128, 1024], bf16, tag="tps",
                                  name=f"tps{it}_{b}")
                    for k in range(8):
                        nc.tensor.transpose(
                            pt[:, k * 128:(k + 1) * 128],
                            ebf[:, (8 * b + k) * 128:(8 * b + k + 1) * 128],
                            id_t)
                    dst = eti[:, 8 * b:8 * b + 8, :]
                    src = pt.rearrange("p (b i) -> p b i", b=8)
                    nc.scalar.activation(
                        dst, src, mybir.ActivationFunctionType.Copy)

                # P@v m-chunks: chunk 0 carries the Z column (same accum
                # group — interleaved psum groups corrupt accumulation on hw)
                AV_CHUNKS = [(1120, 161, True), (0, 224, False),
                             (224, 224, False), (448, 224, False),
                             (672, 224, False), (896, 224, False)]

                def av_mc(it, mc, xr, osb, rz):
                    i0 = it * 128
                    eti = e_ti[it][0]
                    mlo, w, has_z = AV_CHUNKS[mc]
                    wv = w - 1 if has_z else w
                    po = ops.tile([128, 224], f32, tag="ops",
                                  name=f"ops{it}_{mc}")
                    for j in range(NJ):
                        lhs = eti[:, j, :].unsqueeze(1).broadcast_to(
                            [128, 2, 128])
                        nc.tensor.matmul(po[:, 0:w], lhs,
                                         v_t[j][:, :, mlo:mlo + w],
                                         start=(j == 0), stop=(j == NJ - 1),
                                         perf_mode=DR)
                    if has_z:
                        nc.vector.reciprocal(rz, po[:, wv:wv + 1])
                    nc.vector.scalar_tensor_tensor(
                        out=osb[:, mlo:mlo + wv], in0=po[:, 0:wv], scalar=rz,
                        in1=xr[:, mlo:mlo + wv], op0=mybir.AluOpType.mult,
                        op1=mybir.AluOpType.add)
                    if it == NIT - 1:
                        # drain: stream chunks out as they finish
                        nc.sync.dma_start(out=out_d[i0:i0 + 128, mlo:mlo + wv],
                                          in_=osb[:, mlo:mlo + wv])
                    elif mc == 5:
                        nc.sync.dma_start(out=out_d[i0:i0 + 128, :], in_=osb)

                def stage_av_pre(it):
                    if it + 1 < NIT:
                        xr = stgp.tile([128, L], f32, tag="xr",
                                       name=f"xr{it + 1}")
                        nc.sync.dma_start(
                            out=xr, in_=xr_d[(it + 1) * 128:(it + 2) * 128, :])
                        xr_t[it + 1] = xr
                    osb = stgp.tile([128, L], f32, tag="osb", name=f"osb{it}")
                    rz = stgp.tile([128, 1], f32, tag="rz", name=f"rz{it}")
                    return xr_t.pop(it), osb, rz

                xr0 = stgp.tile([128, L], f32, tag="xr", name="xr0")
                nc.sync.dma_start(out=xr0, in_=xr_d[0:128, :])
                xr_t[0] = xr0
                # PE order per block: transposes(vt-2) and P@v(vt-3) m-chunks
                # interleaved (psum-copy latencies hide behind matmuls), S(vt)
                # last; exp(vt-1) emitted after the transpose copies so the
                # ACT queue runs [eti copies, exps] with no head blocking.
                for vt in range(1, NIT + 3):
                    tr = 2 <= vt < NIT + 2
                    av = vt >= 3
                    if 2 <= vt < NIT + 1:
                        stage_exp(vt - 1)
                    if tr:
                        stage_t_batch(vt - 2, 0)
                    ctx = None
                    if av:
                        ctx = stage_av_pre(vt - 3)
                        av_mc(vt - 3, 0, *ctx)
                    if tr:
                        stage_t_batch(vt - 2, 1)
                    if av:
                        av_mc(vt - 3, 1, *ctx)
                    if vt < NIT:
                        stage_s_half(vt, 0)
                    if av:
                        av_mc(vt - 3, 2, *ctx)
                        av_mc(vt - 3, 3, *ctx)
                    if vt < NIT:
                        stage_s_half(vt, 1)
                    if av:
                        av_mc(vt - 3, 4, *ctx)
                        av_mc(vt - 3, 5, *ctx)
                        e_ti.pop(vt - 3)

            pwp.release()
            ebfp.release()

    nc.finalize()
    return nc


_NC = None


def _get_nc():
    global _NC
    if _NC is None:
        _NC = _build()
    return _NC


def kernel(x, Wq, bq, Wk, bk, Wv, bv):
    x = np.asarray(x, dtype=np.float32)
    WqT_full = np.asarray(Wq, np.float32).T                    # [L, H]
    WkT_full = np.asarray(Wk, np.float32).T                    # [L, H]
    def packw(a):  # [L, Hc] -> [128, NL, Hc]: partition-contiguous DMA layout
        return np.ascontiguousarray(
            a.reshape(NL, 128, a.shape[1]).transpose(1, 0, 2))

    WqT = packw(np.ascontiguousarray(WqT_full[:, :H0]))
    WkT = packw(np.ascontiguousarray(WkT_full[:, :H0]))
    Wqk1T = packw(np.ascontiguousarray(
        np.concatenate([WqT_full[:, H0:], WkT_full[:, H0:]], axis=1)))
    WvT = np.asarray(Wv, np.float32).T                         # [L, L]
    bq = np.asarray(bq, np.float32)
    bv = np.asarray(bv, np.float32)

    def pack8(a):  # [L, M] -> [128, NP, 2, M]
        return np.ascontiguousarray(
            a.reshape(NP, 2, 128, a.shape[1]).transpose(2, 0, 1, 3))

    def packw8(a):  # [128, NP, 2, L] -> [MC, 128, NP, 2, 256]
        return np.ascontiguousarray(
            a.reshape(128, NP, 2, MC, 256).transpose(3, 0, 1, 2, 4))

    Wv64 = (32.0 * WvT).astype(np.float32)
    Wh64f = Wv64.astype(E4)
    We64f = (Wv64 - Wh64f.astype(np.float32)).astype(E4)
    Wh64 = packw8(pack8(Wh64f))
    We64 = packw8(pack8(We64f))
    ident = np.eye(128, dtype=ml_dtypes.bfloat16)

    nc = _get_nc()
    in_maps = []
    for b in range(B):
        xT = np.ascontiguousarray(x[b].T)                      # [L, N]
        xh = xT.astype(E4)
        xe = (xT - xh.astype(np.float32)).astype(E4)
        in_maps.append({
            "xT": np.ascontiguousarray(
                xT.reshape(NL, 128, N).transpose(1, 0, 2)),
            "xh": pack8(xh),
            "xe": pack8(xe),
            "WqT": WqT,
            "WkT": WkT,
            "Wqk1T": Wqk1T,
            "Wh64": Wh64,
            "We64": We64,
            "bq": bq,
            "ident": ident,
            "xresid": x[b] + bv[None, :],
        })
    res = run_bass_kernel_spmd(nc, in_maps, list(range(B)))
    return np.stack([res.results[b]["out"] for b in range(B)], axis=0)


if __name__ == "__main__":
    rng = np.random.default_rng(0)
    ins = {
        "x": rng.standard_normal((B, N, L)).astype(np.float32),
        "Wq": rng.standard_normal((H, L)).astype(np.float32) * 0.028,
        "bq": rng.standard_normal((H,)).astype(np.float32) * 0.028,
        "Wk": rng.standard_normal((H, L)).astype(np.float32) * 0.028,
        "bk": rng.standard_normal((H,)).astype(np.float32) * 0.028,
        "Wv": rng.standard_normal((L, L)).astype(np.float32) * 0.028,
        "bv": rng.standard_normal((L,)).astype(np.float32) * 0.028,
    }
    out = kernel(**ins)
    print("kernel ran, out shape", out.shape)
